# revision 4
# baseline (speedup 1.0000x reference)
"""Dense MoE FFN (8 experts, all-expert gating) on 8 TRN2 NeuronCores.

Strategy: data-parallel over tokens. B=8 batches of S=4096 tokens; core b
processes batch b (4096 tokens) with all expert weights replicated, so no
collectives are needed.

Per-core pipeline (everything fp32; matmuls use the float32r full-rate mode):
  - x is fed host-transposed as xT [H, tok].
  - gating:  logitsT = Wg.T @ xT  (feature-major, experts on partitions) ->
             expsT = exp(logitsT + bg)  [E, tok]   (used for the b2 term)
             logits token-major per 128-token tile -> exps_tok [tok, E],
             r = 1 / sum_e exps  [tok, 1]
    (softmax is computed unnormalized; the single 1/sumexp scale is applied
     once at the end.  max-subtraction is skipped: logits are bounded ~|4|.)
  - per expert e:
      mm1 (feature-major): h.T = gelu(W1[e].T @ xT + b1[e])  [F, tok]
          W1 chunks are the stationary operand, b1 rides the ACT bias slot.
      mm2 (token-major): y_e = h @ W2[e]  [tok, H]
          h.T 128-col chunks are stationary; output lands token-major so the
          gate weight exps_tok[:, e] is a per-partition scalar, and the
          accumulation y += exp_e * y_e is ONE fused scalar_tensor_tensor op.
  - b2 term: z = expsT.T @ b2 (K=8 matmul), final y = (yacc + z) * r.
"""

import numpy as np

H, F, E = 256, 512, 8
B, S = 8, 4096
P = 128
TOK = S          # tokens per core (one batch row per core)
KC = H // P      # 2   H chunks
MC = F // P      # 4   F chunks
Q = 4            # token quarters
TQ = TOK // Q    # 1024
NT = TQ // 512   # 2   512-token chunks per quarter (mm1 psum width)
T128 = TQ // P   # 8   128-token tiles per quarter

_CACHE = {}


def _build():
    import concourse.bacc as bacc
    import concourse.mybir as mybir
    import concourse.tile as tile

    f32 = mybir.dt.float32
    f32r = mybir.dt.float32r
    Alu = mybir.AluOpType
    Act = mybir.ActivationFunctionType

    nc = bacc.Bacc("TRN2", target_bir_lowering=False, debug=False)

    xt = nc.dram_tensor("xt", [KC, P, TOK], f32r, kind="ExternalInput")
    wg = nc.dram_tensor("wg", [H, E], f32r, kind="ExternalInput")
    bg = nc.dram_tensor("bg", [1, E], f32, kind="ExternalInput")
    w1 = nc.dram_tensor("w1", [E, KC, P, F], f32r, kind="ExternalInput")
    b1 = nc.dram_tensor("b1", [E * MC, P, 1], f32, kind="ExternalInput")
    w2 = nc.dram_tensor("w2", [E, MC, P, H], f32r, kind="ExternalInput")
    b2 = nc.dram_tensor("b2", [E, H], f32r, kind="ExternalInput")
    y = nc.dram_tensor("y", [TOK, H], f32, kind="ExternalOutput")

    import concourse.bass as bass

    with tile.TileContext(nc) as tc:
        with (
            tc.tile_pool(name="const", bufs=1) as cpool,
            tc.tile_pool(name="gate", bufs=1) as gpool,
            tc.tile_pool(name="gtmp", bufs=4) as gtpool,
            tc.tile_pool(name="h", bufs=8) as hpool,
            tc.tile_pool(name="yacc", bufs=16) as ypool,
            tc.tile_pool(name="out", bufs=4) as opool,
        ):
            # ---- resident inputs ----
            xsb = []
            for kc in range(KC):
                t = cpool.tile([P, TOK], f32r, tag=f"x{kc}")
                for q in range(Q):
                    nc.sync.dma_start(
                        out=t[:, q * TQ:(q + 1) * TQ],
                        in_=xt[kc, :, q * TQ:(q + 1) * TQ],
                    )
                xsb.append(t)
            wgsb = cpool.tile([P, KC * E], f32r, tag="wg")
            for kc in range(KC):
                nc.sync.dma_start(
                    out=wgsb[:, kc * E:(kc + 1) * E],
                    in_=wg[kc * P:(kc + 1) * P, :],
                )
            bgb = cpool.tile([P, E], f32, tag="bgb")
            nc.sync.dma_start(
                out=bgb,
                in_=bass.AP(tensor=bg, offset=0, ap=[[0, P], [1, E]]),
            )
            b2sb = cpool.tile([E, H], f32r, tag="b2")
            nc.sync.dma_start(out=b2sb, in_=b2[:, :])
            b1sb = cpool.tile([P, E * MC], f32, tag="b1")
            for i in range(E * MC):
                nc.sync.dma_start(out=b1sb[:, i:i + 1], in_=b1[i])

            w1sb, w2sb = [], []
            for e in range(E):
                t1 = cpool.tile([P, KC * F], f32r, tag=f"w1_{e}")
                for kc in range(KC):
                    nc.sync.dma_start(
                        out=t1[:, kc * F:(kc + 1) * F], in_=w1[e, kc]
                    )
                w1sb.append(t1)
                t2 = cpool.tile([P, MC * H], f32r, tag=f"w2_{e}")
                for fc in range(MC):
                    nc.sync.dma_start(
                        out=t2[:, fc * H:(fc + 1) * H], in_=w2[e, fc]
                    )
                w2sb.append(t2)

            expsT = cpool.tile([E, TOK], f32r, tag="expsT")
            bgcol = cpool.tile([E, 1], f32, tag="bgcol")
            nc.sync.dma_start(
                out=bgcol, in_=bass.AP(tensor=bg, offset=0, ap=[[1, E], [0, 1]])
            )
            exps_tok = [gpool.tile([P, E], f32, tag=f"exps{i}", name=f"exps{i}") for i in range(S // P)]
            rcol = [gpool.tile([P, 1], f32, tag=f"r{i}", name=f"r{i}") for i in range(S // P)]

            # ---- gating, feature-major: expsT = exp(Wg.T @ xT + bg) ----
            with tc.tile_pool(name="pg1", bufs=2, space="PSUM") as pg1:
                for tc8 in range(TOK // 512):
                    ps = pg1.tile([E, 512], f32, tag="pgT")
                    for kc in range(KC):
                        nc.tensor.matmul(
                            ps,
                            wgsb[:, kc * E:(kc + 1) * E].bitcast(f32r),
                            xsb[kc][:, tc8 * 512:(tc8 + 1) * 512].bitcast(f32r),
                            start=(kc == 0),
                            stop=(kc == KC - 1),
                        )
                    nc.scalar.activation(
                        out=expsT[:, tc8 * 512:(tc8 + 1) * 512],
                        in_=ps,
                        func=Act.Exp,
                        bias=bgcol,
                    )

            # ---- gating, token-major: exps_tok, r = 1/sum ----
            with tc.tile_pool(name="pg2", bufs=4, space="PSUM") as pg2:
                for i in range(S // P):
                    ps = pg2.tile([P, E], f32, tag="pg")
                    for kc in range(KC):
                        nc.tensor.matmul(
                            ps,
                            xsb[kc][:, i * P:(i + 1) * P].bitcast(f32r),
                            wgsb[:, kc * E:(kc + 1) * E].bitcast(f32r),
                            start=(kc == 0),
                            stop=(kc == KC - 1),
                        )
                    lg = gtpool.tile([P, E], f32, tag="lg")
                    nc.vector.tensor_add(lg, ps, bgb)
                    nc.scalar.activation(out=exps_tok[i], in_=lg, func=Act.Exp)
                    sm = gtpool.tile([P, 1], f32, tag="sm")
                    nc.vector.tensor_reduce(
                        sm, exps_tok[i], axis=mybir.AxisListType.X, op=Alu.add
                    )
                    nc.vector.reciprocal(rcol[i], sm)

            # ---- main loop ----
            with (
                tc.tile_pool(name="ps1", bufs=3, space="PSUM") as ps1p,
                tc.tile_pool(name="psy", bufs=3, space="PSUM") as psyp,
                tc.tile_pool(name="psz", bufs=2, space="PSUM") as pszp,
            ):
                for q in range(Q):
                    yq = [ypool.tile([P, H], f32, tag="yacc", name="yacc") for _ in range(T128)]
                    for e in range(E):
                        # mm1: h.T[mc] = gelu(W1[e].T @ xT + b1) over this quarter
                        hq = [hpool.tile([P, TQ], f32r, tag="h", name="h") for _ in range(MC)]
                        for tch in range(NT):
                            for mc in range(MC):
                                ps = ps1p.tile([P, 512], f32, tag="ps1")
                                for kc in range(KC):
                                    nc.tensor.matmul(
                                        ps,
                                        w1sb[e][:, kc * F + mc * P:kc * F + (mc + 1) * P].bitcast(f32r),
                                        xsb[kc][:, q * TQ + tch * 512:q * TQ + (tch + 1) * 512].bitcast(f32r),
                                        start=(kc == 0),
                                        stop=(kc == KC - 1),
                                    )
                                nc.scalar.activation(
                                    out=hq[mc][:, tch * 512:(tch + 1) * 512],
                                    in_=ps,
                                    func=Act.Gelu,
                                    bias=b1sb[:, e * MC + mc:e * MC + mc + 1],
                                )
                        # mm2: y_e = h @ W2[e], gate-scale + accumulate
                        for t8 in range(T128):
                            psy = psyp.tile([P, H], f32, tag="psy")
                            for fc in range(MC):
                                nc.tensor.matmul(
                                    psy,
                                    hq[fc][:, t8 * P:(t8 + 1) * P].bitcast(f32r),
                                    w2sb[e][:, fc * H:(fc + 1) * H].bitcast(f32r),
                                    start=(fc == 0),
                                    stop=(fc == MC - 1),
                                )
                            gcol = exps_tok[q * T128 + t8][:, e:e + 1]
                            if e == 0:
                                nc.vector.tensor_scalar_mul(yq[t8], psy, gcol)
                            else:
                                nc.vector.scalar_tensor_tensor(
                                    yq[t8], psy, gcol, yq[t8],
                                    op0=Alu.mult, op1=Alu.add,
                                )
                    # finalize quarter: y = (yacc + expsT.T @ b2) * r
                    for t8 in range(T128):
                        gi = q * T128 + t8
                        zps = pszp.tile([P, H], f32, tag="z")
                        nc.tensor.matmul(
                            zps,
                            expsT[:, gi * P:(gi + 1) * P].bitcast(f32r),
                            b2sb.bitcast(f32r),
                            start=True,
                            stop=True,
                        )
                        zs = opool.tile([P, H], f32, tag="zs")
                        nc.vector.tensor_scalar_mul(zs, zps, rcol[gi])
                        yo = opool.tile([P, H], f32, tag="yout")
                        nc.vector.scalar_tensor_tensor(
                            yo, yq[t8], rcol[gi], zs, op0=Alu.mult, op1=Alu.add
                        )
                        nc.sync.dma_start(
                            out=y[gi * P:(gi + 1) * P, :], in_=yo
                        )

    nc.compile()
    return nc


def get_nc():
    if "nc" not in _CACHE:
        _CACHE["nc"] = _build()
    return _CACHE["nc"]


def make_in_maps(x, Wg, bg, W1, b1, W2, b2):
    x = np.asarray(x, np.float32)
    in_maps = []
    shared = {
        "wg": np.ascontiguousarray(np.asarray(Wg, np.float32)),
        "bg": np.asarray(bg, np.float32).reshape(1, E),
        "w1": np.ascontiguousarray(np.asarray(W1, np.float32)).reshape(E, KC, P, F),
        "b1": np.ascontiguousarray(np.asarray(b1, np.float32)).reshape(E * MC, P, 1),
        "w2": np.ascontiguousarray(np.asarray(W2, np.float32)).reshape(E, MC, P, H),
        "b2": np.ascontiguousarray(np.asarray(b2, np.float32)),
    }
    for b in range(B):
        m = dict(shared)
        m["xt"] = np.ascontiguousarray(x[b].T).reshape(KC, P, TOK)
        in_maps.append(m)
    return in_maps


def kernel(x, Wg, bg, W1, b1, W2, b2, trace=False):
    from concourse.bass_utils import run_bass_kernel_spmd

    nc = get_nc()
    in_maps = make_in_maps(x, Wg, bg, W1, b1, W2, b2)
    res = run_bass_kernel_spmd(nc, in_maps, core_ids=list(range(B)), trace=trace)
    out = np.stack([res.results[i]["y"] for i in range(B)], axis=0)
    if trace:
        kernel.last_results = res
    return out
